# revision 1
# baseline (speedup 1.0000x reference)
"""Trainium2 Bass kernel for 3-layer GCN + Linear + log_softmax.

nn_GCN3_Lin1: x[100000,16], edge_index[2,6400000] ->
  h = relu(gcn(x;W1,b1)); h = relu(gcn(h;W2,b2)); h = relu(gcn(h;W3,b3))
  out = log_softmax(h @ Wf + bf)

Strategy (8 NeuronCores, graph/data parallel over nodes):
  - Nodes are degree-sorted and dealt round-robin to 8 cores; each core owns
    `slots = n_groups*128` node slots (tail slots are zero dummies).
  - GCN normalization is factored: with u = dinv*h (dinv = rsqrt(1+indeg)),
    out[d] = dinv[d]*(sum_{e:dst=d} u[src_e] + u[d]) + b.  No per-edge float
    work beyond the row sums; the self-loop term is the local row.
  - Per layer: per-128-node-group GEMM on PE -> dinv scale -> shard write ->
    AllGather into a full node-major table in DRAM -> per-group indirect-DMA
    gather of all in-edges (host-built padded index lists; pads point at an
    always-zero row) -> strided tensor_reduce over the padded slot axis ->
    scale/bias/relu -> PE transpose to feed the next GEMM.
  - In-degrees are computed on device from the pad pattern of the index
    lists; host-side preprocessing is pure integer index manipulation.
"""

import math

import numpy as np

from concourse import bass, mybir, bacc, tile
from concourse.bass_utils import run_bass_kernel_spmd
from concourse.masks import make_identity

F32 = mybir.dt.float32
I32 = mybir.dt.int32
GROUP = 128
N_CORES = 8
DIMS = (16, 32, 24, 12, 6)

LAST_RUN_INFO = {}


# ---------------------------------------------------------------------------
# Host-side plan (pure index manipulation)
# ---------------------------------------------------------------------------
class _Plan:
    pass


def _build_plan(edge_index, n_nodes, n_cores=N_CORES):
    src = np.asarray(edge_index[0], dtype=np.int64)
    dst = np.asarray(edge_index[1], dtype=np.int64)
    assert n_nodes % n_cores == 0
    per_core = n_nodes // n_cores
    n_groups = math.ceil(per_core / GROUP)
    slots = n_groups * GROUP
    assert per_core < slots, "need at least one dummy slot for the zero/pad row"

    deg_in = np.bincount(dst, minlength=n_nodes).astype(np.int64)

    order = np.argsort(-deg_in, kind="stable")
    ranks = np.empty(n_nodes, dtype=np.int64)
    ranks[order] = np.arange(n_nodes)
    core_of = ranks % n_cores
    q_of = ranks // n_cores
    g_of = q_of // GROUP
    p_of = q_of % GROUP
    slot_of = p_of * n_groups + g_of
    table_row = core_of * slots + slot_of

    Pg = np.zeros(n_groups, dtype=np.int64)
    np.maximum.at(Pg, g_of, deg_in)
    Pg = np.maximum(2, ((Pg + 1) // 2) * 2).astype(np.int64)
    off = np.zeros(n_groups + 1, dtype=np.int64)
    off[1:] = np.cumsum(Pg)
    S_tot = int(off[-1])

    PAD_ROW = slots - 1

    # int32 padded lists (p-major slots) — used only for on-device degree calc
    idx_all = np.full((n_cores, GROUP, S_tot), PAD_ROW, dtype=np.int32)
    dst_core = core_of[dst]
    dst_q = q_of[dst]
    src_row = table_row[src].astype(np.int32)
    for c in range(n_cores):
        m = dst_core == c
        q_c = dst_q[m]
        s_c = src_row[m]
        o = np.argsort(q_c, kind="stable")
        q_s = q_c[o]
        s_s = s_c[o]
        node_start = np.searchsorted(q_s, np.arange(per_core))
        j = np.arange(len(q_s)) - node_start[q_s]
        g = q_s // GROUP
        p = q_s % GROUP
        idx_all[c][p, off[g] + j] = s_s

    # ---- int16 call plan for dma_gather (Ant ucode) ----
    # sub-tables of `cps` cores each so local rows fit int16; zero/pad row of
    # sub-table b is its first core's dummy slot (local row slots-1)
    cps = max(1, 32768 // slots)
    cps = min(cps, n_cores)
    n_sub = math.ceil(n_cores / cps)
    sub_rows = cps * slots
    assert sub_rows - (cps - 1) * slots - 1 <= 32767

    src_core = core_of[src]
    src_sub = src_core // cps
    src_local = table_row[src] - src_sub * sub_rows

    # per (group, sub) padded slot count, max over cores
    cnt_gs = np.zeros((n_cores, per_core, n_sub), dtype=np.int32)
    np.add.at(cnt_gs, (dst_core, dst_q, src_sub), 1)
    group_of_q = np.repeat(np.arange(n_groups), GROUP)[:per_core]
    Pgs = np.zeros((n_groups, n_sub), dtype=np.int64)
    for b in range(n_sub):
        tmp = np.zeros(n_groups, dtype=np.int64)
        np.maximum.at(tmp, group_of_q, cnt_gs[:, :, b].max(axis=0))
        Pgs[:, b] = tmp
    for g in range(n_groups):
        if Pgs[g].sum() == 0:
            Pgs[g, 0] = 1  # all-pad call keeps the reduce well-defined

    # calls: (g, b, cnt) chunks of <=64 slots (num_idxs <= 8192 per ucode call)
    CH = 64
    calls = []  # (g, b, slot_off_in_group_tile, cnt, col16_off)
    slot_tot = np.zeros(n_groups, dtype=np.int64)
    col16 = 0
    for g in range(n_groups):
        so = 0
        for b in range(n_sub):
            s0 = 0
            while s0 < int(Pgs[g, b]):
                cnt = min(CH, int(Pgs[g, b]) - s0)
                calls.append((g, b, so, cnt, col16))
                col16 += cnt * 8
                so += cnt
                s0 += cnt
        slot_tot[g] = so
    TOT16 = col16

    # per-core wrapped int16 index arrays (idx i of a call -> [i%16 (+16r), i//16])
    PAD_LOCAL = slots - 1  # sub-table's first core's dummy slot (always zero)
    idx16_all = np.full((n_cores, GROUP, TOT16), PAD_LOCAL, dtype=np.int16)
    base = np.full((n_groups, n_sub, 16), -1, dtype=np.int64)
    for g, b, so, cnt, c16 in calls:
        ci = 0
        while base[g, b, ci] >= 0:
            ci += 1
        base[g, b, ci] = c16
    for c in range(n_cores):
        m = dst_core == c
        q_c = dst_q[m]
        b_c = src_sub[m]
        l_c = src_local[m]
        o = np.argsort(q_c * n_sub + b_c, kind="stable")
        q_s = q_c[o]
        b_s = b_c[o]
        l_s = l_c[o].astype(np.int16)
        key = q_s * n_sub + b_s
        starts = np.searchsorted(key, np.arange(per_core * n_sub))
        j = np.arange(len(key)) - starts[key]
        g_s = q_s // GROUP
        p_s = q_s % GROUP
        chunk = j // CH
        s_in = j % CH
        c16_e = base[g_s, b_s, chunk]
        assert (c16_e >= 0).all()
        i_flat = s_in * GROUP + p_s
        col = c16_e + i_flat // 16
        prow = i_flat % 16
        for rep in range(8):
            idx16_all[c][prow + rep * 16, col] = l_s

    pl = _Plan()
    pl.n_cores = n_cores
    pl.n_nodes = n_nodes
    pl.per_core = per_core
    pl.n_groups = n_groups
    pl.slots = slots
    pl.Pg = Pg
    pl.off = off
    pl.S_tot = S_tot
    pl.PAD_ROW = PAD_ROW
    pl.idx_all = idx_all
    pl.cps = cps
    pl.n_sub = n_sub
    pl.sub_rows = sub_rows
    pl.calls = calls
    pl.slot_tot = slot_tot
    pl.TOT16 = TOT16
    pl.idx16_all = idx16_all
    pl.core_of = core_of
    pl.q_of = q_of
    pl.g_of = g_of
    pl.p_of = p_of
    return pl


def _make_in_maps(pl, x, W1, b1, W2, b2, W3, b3, Wf, bf):
    d_in = x.shape[1]
    in_maps = []
    for c in range(pl.n_cores):
        own = pl.core_of == c
        xT = np.zeros((d_in, pl.slots), dtype=np.float32)
        xT[:, pl.q_of[own]] = x[own].T
        in_maps.append(
            {
                "xT_in": xT,
                "idx_in": np.ascontiguousarray(pl.idx_all[c]),
                "idx16_in": np.ascontiguousarray(pl.idx16_all[c]),
                "W1": np.asarray(W1, np.float32),
                "W2": np.asarray(W2, np.float32),
                "W3": np.asarray(W3, np.float32),
                "Wf": np.asarray(Wf, np.float32),
                "b1": np.tile(np.asarray(b1, np.float32)[None, :], (GROUP, 1)),
                "b2": np.tile(np.asarray(b2, np.float32)[None, :], (GROUP, 1)),
                "b3": np.tile(np.asarray(b3, np.float32)[None, :], (GROUP, 1)),
                "bf": np.tile(np.asarray(bf, np.float32)[None, :], (GROUP, 1)),
            }
        )
    return in_maps


def _assemble_output(pl, outs_per_core, d_out):
    full = np.empty((pl.n_nodes, d_out), dtype=np.float32)
    rows = pl.p_of * pl.n_groups + pl.g_of
    for c in range(pl.n_cores):
        own = pl.core_of == c
        full[own] = outs_per_core[c][rows[own]]
    return full


# ---------------------------------------------------------------------------
# Device kernel
# ---------------------------------------------------------------------------
def _build_kernel(pl, dims=DIMS, gather_bufs=3, debug=False):
    d0, d1, d2, d3, d4 = dims
    NG, S, SL = pl.n_groups, pl.S_tot, pl.slots
    NC = pl.n_cores
    Pg, off = pl.Pg, pl.off
    dmax = max(d1, d2, d3)

    nc = bacc.Bacc("TRN2", target_bir_lowering=False, debug=False, num_devices=NC)

    xT_in = nc.dram_tensor("xT_in", [d0, SL], F32, kind="ExternalInput")
    idx_in = nc.dram_tensor("idx_in", [GROUP, S], I32, kind="ExternalInput")
    idx16_in = nc.dram_tensor(
        "idx16_in", [GROUP, pl.TOT16], mybir.dt.int16, kind="ExternalInput"
    )
    wdims = {"W1": (d0, d1), "W2": (d1, d2), "W3": (d2, d3), "Wf": (d3, d4)}
    bdims = {"b1": d1, "b2": d2, "b3": d3, "bf": d4}
    Ws = {
        n: nc.dram_tensor(n, list(ab), F32, kind="ExternalInput")
        for n, ab in wdims.items()
    }
    bs = {
        n: nc.dram_tensor(n, [GROUP, d], F32, kind="ExternalInput")
        for n, d in bdims.items()
    }
    out_dram = nc.dram_tensor("out", [SL, d4], F32, kind="ExternalOutput")
    if debug:
        dbg_dinv = nc.dram_tensor("dbg_dinv", [GROUP, NG], F32, kind="ExternalOutput")
        dbg_hown1 = nc.dram_tensor("dbg_hown1", [GROUP, NG * dims[1]], F32, kind="ExternalOutput")
        dbg_tab = nc.dram_tensor("dbg_tab", [2 * SL, 64], F32, kind="ExternalOutput")
        dbg_z1 = nc.dram_tensor("dbg_z1", [GROUP, NG * dims[1]], F32, kind="ExternalOutput")

    EL = 64  # table row width (256B rows for the Ant gather ucode)
    shard = {}
    table = {}
    for k in (1, 2, 3):
        shard[k] = nc.dram_tensor(f"shard{k}", [SL, EL], F32)
        table[k] = nc.dram_tensor(f"table{k}", [NC * SL, EL], F32, addr_space="Shared")

    rgroups = [list(range(NC))]

    with tile.TileContext(nc, num_cores=NC) as tc:
        with (
            tc.tile_pool(name="persist", bufs=1) as pers,
            tc.tile_pool(name="gat", bufs=gather_bufs) as gpool,
            tc.tile_pool(name="idx16", bufs=6) as ipool,
            tc.tile_pool(name="work", bufs=4) as wpool,
            tc.tile_pool(name="ps", bufs=4, space="PSUM") as ppool,
            tc.tile_pool(name="pst", bufs=4, space="PSUM") as ppool2,
        ):
            idx_sb = gpool.tile([GROUP, S], I32, name="idx_sb", tag="gat")
            nc.sync.dma_start(idx_sb[:], idx_in[:, :])
            # pre-zero the 64-wide shard bounce buffers (pad cols stay zero)
            zsb = gpool.tile([GROUP, SL * EL // GROUP], F32, name="zsb", tag="gat")
            nc.vector.memset(zsb[:], 0.0)
            for k in (1, 2, 3):
                nc.sync.dma_start(
                    shard[k].ap().rearrange("(p r) f -> p (r f)", p=GROUP), zsb[:]
                )
            xT_sb = pers.tile([dmax, SL], F32)
            nc.sync.dma_start(xT_sb[:d0, :], xT_in[:, :])
            W_sb = {}
            for n, ab in wdims.items():
                W_sb[n] = pers.tile(list(ab), F32, name=f"Wsb_{n}")
                nc.sync.dma_start(W_sb[n][:], Ws[n][:, :])
            b_sb = {}
            for n, d in bdims.items():
                b_sb[n] = pers.tile([GROUP, d], F32, name=f"bsb_{n}")
                nc.sync.dma_start(b_sb[n][:], bs[n][:, :])

            ident = pers.tile([GROUP, GROUP], F32)
            make_identity(nc, ident[:])

            # ---- in-degree -> dinv [128, NG] (from pad pattern) ----
            idx_f = gpool.tile([GROUP, S], F32, name="idx_f", tag="gat")
            nc.vector.tensor_copy(idx_f[:], idx_sb[:])
            ispad = gpool.tile([GROUP, S], F32, name="ispad", tag="gat")
            nc.vector.tensor_scalar(
                out=ispad[:],
                in0=idx_f[:],
                scalar1=float(pl.PAD_ROW),
                scalar2=None,
                op0=mybir.AluOpType.is_equal,
            )
            deg = pers.tile([GROUP, NG], F32)
            padcnt = pers.tile([GROUP, NG], F32)
            for g in range(NG):
                nc.vector.tensor_reduce(
                    out=padcnt[:, g : g + 1],
                    in_=ispad[:, off[g] : off[g + 1]],
                    axis=mybir.AxisListType.X,
                    op=mybir.AluOpType.add,
                )
                nc.vector.tensor_scalar(
                    out=deg[:, g : g + 1],
                    in0=padcnt[:, g : g + 1],
                    scalar1=-1.0,
                    scalar2=float(Pg[g] + 1),
                    op0=mybir.AluOpType.mult,
                    op1=mybir.AluOpType.add,
                )
            dinv = pers.tile([GROUP, NG], F32)
            nc.vector.reciprocal(dinv[:], deg[:])
            nc.scalar.activation(
                out=dinv[:], in_=dinv[:], func=mybir.ActivationFunctionType.Sqrt
            )

            hown = pers.tile([GROUP, NG * dmax], F32)
            out_sb = pers.tile([GROUP, NG * d4], F32)
            n_dummy_p = SL - pl.per_core

            for k, din, dout, wname, bname in [
                (1, d0, d1, "W1", "b1"),
                (2, d1, d2, "W2", "b2"),
                (3, d2, d3, "W3", "b3"),
            ]:
                # h'own = dinv * (x @ W)
                for g in range(NG):
                    ps = ppool.tile([GROUP, dout], F32, space="PSUM", name=f"ps_{k}_{g}", tag="ps")
                    nc.tensor.matmul(
                        ps[:],
                        lhsT=xT_sb[:din, g * GROUP : (g + 1) * GROUP],
                        rhs=W_sb[wname][:],
                        start=True,
                        stop=True,
                    )
                    nc.vector.tensor_scalar_mul(
                        hown[:, g * dout : (g + 1) * dout], ps[:], dinv[:, g : g + 1]
                    )
                nc.sync.dma_start(
                    shard[k][:, :dout].rearrange("(p g) f -> p g f", g=NG),
                    hown[:, : NG * dout],
                )
                nc.gpsimd.collective_compute(
                    "AllGather",
                    mybir.AluOpType.bypass,
                    replica_groups=rgroups,
                    ins=[shard[k].ap().opt()],
                    outs=[table[k].ap().opt()],
                )
                if debug and k == 1:
                    nc.sync.dma_start(dbg_dinv[:, :], dinv[:])
                    nc.sync.dma_start(dbg_hown1[:, :], hown[:, : NG * dout])
                    tcp = wpool.tile(
                        [GROUP, 2 * SL * 64 // GROUP], F32, name="tcp", tag="tcp", bufs=1
                    )
                    nc.sync.dma_start(
                        tcp[:],
                        table[k][0 : 2 * SL, :].rearrange(
                            "(c p g) f -> p c g f", c=2, p=GROUP
                        ),
                    )
                    nc.sync.dma_start(
                        dbg_tab.ap().rearrange("(c p g) f -> p c g f", c=2, p=GROUP),
                        tcp[:],
                    )
                ci = 0
                for g in range(NG):
                    stot = int(pl.slot_tot[g])
                    gat = gpool.tile(
                        [GROUP, stot * EL], F32, name=f"gat_{k}_{g}", tag="gat"
                    )
                    while ci < len(pl.calls) and pl.calls[ci][0] == g:
                        _, b, so, cnt, c16 = pl.calls[ci]
                        it = ipool.tile(
                            [GROUP, cnt * 8],
                            mybir.dt.int16,
                            name=f"it_{k}_{ci}",
                            tag="it",
                        )
                        nc.sync.dma_start(it[:], idx16_in[:, c16 : c16 + cnt * 8])
                        nc.gpsimd.dma_gather(
                            out_ap=gat[:, so * EL : (so + cnt) * EL].rearrange(
                                "p (s f) -> p s f", f=EL
                            ),
                            in_ap=table[k][b * pl.sub_rows : (b + 1) * pl.sub_rows, :],
                            idxs_ap=it[:],
                            num_idxs=cnt * GROUP,
                            num_idxs_reg=cnt * GROUP,
                            elem_size=EL,
                            single_packet=False,
                        )
                        ci += 1
                    z = wpool.tile([GROUP, dout], F32, name=f"z_{k}_{g}", tag="z")
                    nc.vector.tensor_reduce(
                        out=z[:],
                        in_=gat[:].rearrange("p (s f) -> p f s", f=EL)[:, :dout, :],
                        axis=mybir.AxisListType.X,
                        op=mybir.AluOpType.add,
                    )
                    if debug and k == 1:
                        nc.sync.dma_start(
                            dbg_z1[:, g * dout : (g + 1) * dout], z[:]
                        )
                    nc.vector.tensor_add(z[:], z[:], hown[:, g * dout : (g + 1) * dout])
                    nc.vector.scalar_tensor_tensor(
                        out=z[:],
                        in0=z[:],
                        scalar=dinv[:, g : g + 1],
                        in1=b_sb[bname][:],
                        op0=mybir.AluOpType.mult,
                        op1=mybir.AluOpType.add,
                    )
                    rl = wpool.tile([GROUP, dout], F32, name=f"rl_{k}_{g}", tag="rl")
                    nc.scalar.activation(
                        out=rl[:], in_=z[:], func=mybir.ActivationFunctionType.Relu
                    )
                    pst = ppool2.tile(
                        [dout, GROUP], F32, space="PSUM", name=f"pst_{k}_{g}", tag="pst"
                    )
                    nc.tensor.transpose(out=pst[:], in_=rl[:], identity=ident[:])
                    nc.vector.tensor_copy(
                        xT_sb[:dout, g * GROUP : (g + 1) * GROUP], pst[:]
                    )
                nc.vector.memset(xT_sb[:dout, SL - n_dummy_p : SL], 0.0)

            # ---- final linear + log_softmax ----
            for g in range(NG):
                ps = ppool.tile([GROUP, d4], F32, space="PSUM", name=f"psf_{g}", tag="ps")
                nc.tensor.matmul(
                    ps[:],
                    lhsT=xT_sb[:d3, g * GROUP : (g + 1) * GROUP],
                    rhs=W_sb["Wf"][:],
                    start=True,
                    stop=True,
                )
                logits = wpool.tile([GROUP, d4], F32, name=f"lg_{g}", tag="lg")
                nc.vector.tensor_add(logits[:], ps[:], b_sb["bf"][:])
                m = wpool.tile([GROUP, 1], F32, name=f"m_{g}", tag="m")
                nc.vector.tensor_reduce(
                    out=m[:],
                    in_=logits[:],
                    axis=mybir.AxisListType.X,
                    op=mybir.AluOpType.max,
                )
                negm = wpool.tile([GROUP, 1], F32, name=f"nm_{g}", tag="nm")
                nc.vector.tensor_scalar_mul(negm[:], m[:], -1.0)
                e = wpool.tile([GROUP, d4], F32, name=f"e_{g}", tag="e")
                s = wpool.tile([GROUP, 1], F32, name=f"s_{g}", tag="s")
                nc.scalar.activation(
                    out=e[:],
                    in_=logits[:],
                    func=mybir.ActivationFunctionType.Exp,
                    bias=negm[:],
                    scale=1.0,
                    accum_out=s[:],
                )
                ls = wpool.tile([GROUP, 1], F32, name=f"ls_{g}", tag="ls")
                nc.scalar.activation(
                    out=ls[:], in_=s[:], func=mybir.ActivationFunctionType.Ln
                )
                shift = wpool.tile([GROUP, 1], F32, name=f"sh_{g}", tag="sh")
                nc.vector.tensor_sub(shift[:], negm[:], ls[:])
                nc.vector.tensor_scalar_add(
                    out_sb[:, g * d4 : (g + 1) * d4], logits[:], shift[:]
                )

            nc.sync.dma_start(
                out_dram.ap().rearrange("(p g) f -> p (g f)", g=NG),
                out_sb[:],
            )

    nc.compile()
    return nc


# ---------------------------------------------------------------------------
# Entry point
# ---------------------------------------------------------------------------
def kernel(x, edge_index, W1, b1, W2, b2, W3, b3, Wf, bf):
    x = np.asarray(x, dtype=np.float32)
    n_nodes = x.shape[0]
    pl = _build_plan(np.asarray(edge_index), n_nodes)
    nc = _build_kernel(pl)
    in_maps = _make_in_maps(pl, x, W1, b1, W2, b2, W3, b3, Wf, bf)

    res = run_bass_kernel_spmd(nc, in_maps, core_ids=list(range(pl.n_cores)))

    LAST_RUN_INFO.clear()
    LAST_RUN_INFO["exec_time_ns"] = res.exec_time_ns
    LAST_RUN_INFO["mean_exec_time_ns"] = res.mean_exec_time_ns

    outs = [res.results[c]["out"] for c in range(pl.n_cores)]
    return _assemble_output(pl, outs, d_out=DIMS[-1])

